# revision 13
# baseline (speedup 1.0000x reference)
"""Trainium2 Bass kernel for nn_ANA2B (per-pair multipole interaction network).

Self-contained: host-side sharding/index-prep (numpy) + Bass/Tile kernel run
SPMD on 8 NeuronCores via PJRT. Strategy:
  - pairs data-parallel over 8 cores (32768 pairs/core, lex-sorted input order)
  - tiny GNN + per-atom femb-L1 projections computed on-device per core
  - femb layer 1 evaluated as one-hot matmuls (no explicit H gather)
  - per-atom multipole/coord tables gathered by ap_gather (8 Q7 groups =
    2 tables x 4 pair-slices), PE-transposed into pairs-on-partitions layout
  - G-matrix / distances / switch / V assembly in pairs-on-partitions layout
  - MLPs in feature-major bf16 with ACT-table-set-batched activations
  - per-dimer segment sum via one-hot-b matmuls into a PSUM accumulator;
    host sums the 8 per-core partials
"""
import sys
sys.path.insert(0, '/opt/trn_rl_repo')
import numpy as np
import ml_dtypes

BF = ml_dtypes.bfloat16
Pc = 32768          # pairs per core
SC = 16384          # superchunk
NSC = Pc // SC      # 2
NSLICE = 4
SLICE = SC // NSLICE   # 4096
NG = Pc // 128         # 256 gammas
NGS = SC // 128        # 128 gammas per superchunk
NCHS = SC // 512       # 32 chunks per superchunk

GAMMAS = np.logspace(-1, 0, 5).astype(np.float32)

_RUNNER = None


# ---------------------------------------------------------------------------
# host prep
# ---------------------------------------------------------------------------

def _host_prep(inputs):
    p = inputs['params']
    g = p['gnn']
    np_ = np.asarray

    ib = np_(inputs['indices_b']).astype(np.int64)
    ii = np_(inputs['indices_i']).astype(np.int64)
    ij = np_(inputs['indices_j']).astype(np.int64)
    assert len(ib) == 8 * Pc

    def t_table(mono, dip, quad, coord):
        B, n = np_(mono).shape[:2]
        t = np.concatenate([
            np_(mono).reshape(B, n, 1), np_(dip).reshape(B, n, 3),
            np_(quad).reshape(B, n, 9), np_(coord).reshape(B, n, 3)], axis=-1)
        return t.reshape(B * n, 16).astype(np.float32)

    T1 = t_table(inputs['monopoles_1'], inputs['dipoles_1'],
                 inputs['quadrupoles_1'], inputs['coords_1'])
    T2 = t_table(inputs['monopoles_2'], inputs['dipoles_2'],
                 inputs['quadrupoles_2'], inputs['coords_2'])
    ttab = np.zeros((128, 6144), np.float32)
    for l in range(4):
        ttab[32*l:32*l+16, :] = T1.T
        ttab[32*l+16:32*l+32, :] = T2.T

    nodes1T = np_(inputs['nodes_1']).T.astype(BF)
    nodes2T = np_(inputs['nodes_2']).T.astype(BF)
    ef1T = np_(inputs['edge_feats_1']).T.astype(BF)
    ef2T = np_(inputs['edge_feats_2']).T.astype(BF)
    e1 = np_(inputs['edge_index_1'])
    e2 = np_(inputs['edge_index_2'])
    src1_rep = np.tile(e1[0].astype(np.float16)[None, :], (128, 1))
    src2_rep = np.tile(e2[0].astype(np.float16)[None, :], (128, 1))
    dst1_col = e1[1].astype(np.float32).reshape(8, 128).T.copy()
    dst2_col = e2[1].astype(np.float32).reshape(8, 128).T.copy()

    def W(x):
        return np_(x).astype(BF)

    def bcol(x):
        return np_(x).astype(np.float32).reshape(-1, 1)

    femb, S, K = p['femb'], p['S'], p['K']
    SW1 = np_(S[0][0]).astype(np.float32)
    KW1 = np_(K[0][0]).astype(np.float32)
    SW1_G = np.zeros((16, 128), np.float32)
    SW1_G[0:10] = SW1[0:10]
    SW1_G[10:15] = SW1[138:143]
    KW1_G = np.zeros((16, 128), np.float32)
    KW1_G[10:15] = KW1[128:133]

    shared = dict(
        ttab=ttab,
        nodes1T=nodes1T, nodes2T=nodes2T, ef1T=ef1T, ef2T=ef2T,
        src1_rep=src1_rep, src2_rep=src2_rep,
        dst1_col=dst1_col, dst2_col=dst2_col,
        gnn_embed_W=W(g['embed_W']), gnn_embed_b=bcol(g['embed_b']),
        gnn_msg_W1h=W(np_(g['msg_W1'])[0:64]), gnn_msg_W1e=W(np_(g['msg_W1'])[64:72]), gnn_msg_b1=bcol(g['msg_b1']),
        gnn_msg_W2=W(g['msg_W2']), gnn_msg_b2=bcol(g['msg_b2']),
        gnn_upd_Wh=W(np_(g['upd_W'])[0:64]), gnn_upd_Wa=W(np_(g['upd_W'])[64:128]), gnn_upd_b=bcol(g['upd_b']),
        femb_W1a=W(np_(femb[0][0])[0:64]), femb_W1b=W(np_(femb[0][0])[64:128]), femb_b1=bcol(femb[0][1]),
        femb_W2=W(femb[1][0]), femb_b2=bcol(femb[1][1]),
        femb_W3=W(femb[2][0]), femb_b3=bcol(femb[2][1]) * 2.0,
        S_W1_nf=W(SW1[10:138]), S_W1_G=W(SW1_G), S_b1=bcol(S[0][1]),
        S_W2=W(S[1][0]), S_b2=bcol(S[1][1]),
        S_W3=W(S[2][0]), S_b3=bcol(S[2][1]),
        K_W1_nf=W(KW1[0:128]), K_W1_G=W(KW1_G), K_b1=bcol(K[0][1]),
        K_W2=W(K[1][0]), K_b2=bcol(K[1][1]),
        K_W3=W(K[2][0]), K_b3=bcol(K[2][1]),
    )

    cores = []
    for c in range(8):
        sl = slice(c * Pc, (c + 1) * Pc)
        cib, cii, cij = ib[sl], ii[sl], ij[sl]
        u1 = (cib * 384 + cii)
        u2 = (cib * 384 + cij)
        tidx = np.zeros((128, (SLICE // 16) * NSC), np.int16)
        for s in range(NSC):
            for l in range(NSLICE):
                q0 = s * SC + l * SLICE
                u1s = u1[q0:q0 + SLICE].reshape(SLICE // 16, 16)
                u2s = u2[q0:q0 + SLICE].reshape(SLICE // 16, 16)
                cols = slice(s * (SLICE // 16), (s + 1) * (SLICE // 16))
                tidx[32*l:32*l+16, cols] = u1s.T
                tidx[32*l+16:32*l+32, cols] = u2s.T
        cores.append(dict(
            shared,
            tidx=tidx,
            ii_rep=np.tile(cii.astype(np.float16)[None, :], (128, 1)),
            ij_rep=np.tile(cij.astype(np.float16)[None, :], (128, 1)),
            ib_scal=cib.astype(np.float32).reshape(NG, 128).T.copy(),
        ))
    return cores


# ---------------------------------------------------------------------------
# bass kernel
# ---------------------------------------------------------------------------

def _build_nc():
    from concourse import bass, bacc, mybir, tile
    from concourse.masks import make_identity

    F32 = mybir.dt.float32
    BF16 = mybir.dt.bfloat16
    I16 = mybir.dt.int16
    I32 = mybir.dt.int32
    AF = mybir.ActivationFunctionType
    ALU = mybir.AluOpType
    AX = mybir.AxisListType

    nc = bacc.Bacc("TRN2", target_bir_lowering=False, debug=False, num_devices=8)

    def din(name, shape, dt):
        return nc.dram_tensor(name, shape, dt, kind="ExternalInput")

    ttab = din("ttab", [128, 6144], F32)
    tidx = din("tidx", [128, (SLICE // 16) * NSC], I16)
    FP16 = mybir.dt.float16
    ii_rep = din("ii_rep", [128, Pc], FP16)
    ij_rep = din("ij_rep", [128, Pc], FP16)
    ib_scal = din("ib_scal", [128, NG], F32)
    nodesT = {1: din("nodes1T", [32, 384], BF16), 2: din("nodes2T", [32, 384], BF16)}
    efT = {1: din("ef1T", [8, 1024], BF16), 2: din("ef2T", [8, 1024], BF16)}
    src_rep = {1: din("src1_rep", [128, 1024], FP16), 2: din("src2_rep", [128, 1024], FP16)}
    dst_col = {1: din("dst1_col", [128, 8], F32), 2: din("dst2_col", [128, 8], F32)}
    wnames = ["gnn_embed_W", "gnn_msg_W1h", "gnn_msg_W1e", "gnn_msg_W2",
              "gnn_upd_Wh", "gnn_upd_Wa",
              "femb_W1a", "femb_W1b", "femb_W2", "femb_W3",
              "S_W1_nf", "S_W1_G", "S_W2", "S_W3",
              "K_W1_nf", "K_W1_G", "K_W2", "K_W3"]
    wshapes = {"gnn_embed_W": [32, 64], "gnn_msg_W1h": [64, 64], "gnn_msg_W1e": [8, 64],
               "gnn_msg_W2": [64, 64],
               "gnn_upd_Wh": [64, 64], "gnn_upd_Wa": [64, 64],
               "femb_W1a": [64, 128], "femb_W1b": [64, 128], "femb_W2": [128, 128],
               "femb_W3": [128, 128], "S_W1_nf": [128, 128], "S_W1_G": [16, 128],
               "S_W2": [128, 128], "S_W3": [128, 2], "K_W1_nf": [128, 128],
               "K_W1_G": [16, 128], "K_W2": [128, 128], "K_W3": [128, 3]}
    bnames = ["gnn_embed_b", "gnn_msg_b1", "gnn_msg_b2", "gnn_upd_b",
              "femb_b1", "femb_b2", "femb_b3", "S_b1", "S_b2", "S_b3",
              "K_b1", "K_b2", "K_b3"]
    bshapes = {"gnn_embed_b": 64, "gnn_msg_b1": 64, "gnn_msg_b2": 64, "gnn_upd_b": 64,
               "femb_b1": 128, "femb_b2": 128, "femb_b3": 128, "S_b1": 128,
               "S_b2": 128, "S_b3": 2, "K_b1": 128, "K_b2": 128, "K_b3": 3}
    wd = {n: din(n, wshapes[n], BF16) for n in wnames}
    bd = {n: din(n, [bshapes[n], 1], F32) for n in bnames}

    out_d = nc.dram_tensor("out", [1, 16], F32, kind="ExternalOutput")

    from contextlib import ExitStack
    with tile.TileContext(nc) as tc:
        with (
            tc.tile_pool(name="const", bufs=1) as cp,
            tc.tile_pool(name="work", bufs=2) as wp,
            tc.tile_pool(name="idx", bufs=2) as ip,
            tc.tile_pool(name="ps2", bufs=2, space="PSUM") as pp,
            tc.tile_pool(name="ps1", bufs=1, space="PSUM") as pp1,
        ):
            # ---------------- constants ----------------
            ident = cp.tile([128, 128], F32)
            make_identity(nc, ident[:])
            identb = cp.tile([128, 128], BF16)
            nc.vector.tensor_copy(identb[:], ident[:])
            iota_col = cp.tile([128, 1], I32)
            nc.gpsimd.iota(iota_col[:], pattern=[[0, 1]], base=0, channel_multiplier=1)
            iota_colf = cp.tile([128, 1], F32)
            nc.vector.tensor_copy(iota_colf[:], iota_col[:])
            iota16 = cp.tile([128, 16], I32)
            nc.gpsimd.iota(iota16[:], pattern=[[1, 16]], base=0, channel_multiplier=0)
            iota16f = cp.tile([128, 16], F32)
            nc.vector.tensor_copy(iota16f[:], iota16[:])


            ttab_t = cp.tile([128, 6144], F32)
            nc.sync.dma_start(ttab_t[:], ttab[:])
            tidx_t = cp.tile([128, (SLICE // 16) * NSC], I16)
            nc.sync.dma_start(tidx_t[:], tidx[:])
            ib_t = cp.tile([128, NG], F32)
            nc.sync.dma_start(ib_t[:], ib_scal[:])

            w = {}
            for n in wnames:
                w[n] = cp.tile(wshapes[n], BF16, tag=f"w_{n}", name=f"w_{n}")
                nc.sync.dma_start(w[n][:], wd[n][:])
            b = {}
            for n in bnames:
                b[n] = cp.tile([bshapes[n], 1], F32, tag=f"b_{n}", name=f"b_{n}")
                nc.sync.dma_start(b[n][:], bd[n][:])


            # ---------------- GNN (per side) ----------------
            gnn_stack = ExitStack()
            gwp = gnn_stack.enter_context(tc.tile_pool(name="gnnp", bufs=1))
            iota384 = gwp.tile([128, 384], I32)
            nc.gpsimd.iota(iota384[:], pattern=[[1, 384]], base=0, channel_multiplier=0)
            iota384f = gwp.tile([128, 384], F32)
            nc.vector.tensor_copy(iota384f[:], iota384[:])
            gnn_in = {}
            for sd in (1, 2):
                gnn_in[sd] = {}
                for nm, dt_, shp in (("nodesT", BF16, [32, 384]), ("efT", BF16, [8, 1024]),
                                     ("src_rep", FP16, [128, 1024]), ("dst_col", F32, [128, 8])):
                    t = gwp.tile(shp, dt_, tag=f"gnn_{nm}{sd}", name=f"gnn_{nm}{sd}")
                    srcd = {"nodesT": nodesT, "efT": efT, "src_rep": src_rep,
                            "dst_col": dst_col}[nm][sd]
                    nc.sync.dma_start(t[:], srcd[:])
                    gnn_in[sd][nm] = t

            def run_gnn(sd):
                gi = gnn_in[sd]
                ps_h = pp1.tile([64, 384], F32, tag="aux")
                nc.tensor.matmul(ps_h[:], w["gnn_embed_W"][:], gi["nodesT"][:],
                                 start=True, stop=True)
                h0 = gwp.tile([64, 384], BF16, tag=f"h0_{sd}")
                nc.scalar.activation(h0[:], ps_h[:], AF.Silu, bias=b["gnn_embed_b"][:])

                # h_vf blocks [128, 64] x3 (transpose of h0)
                hvf = gwp.tile([128, 3 * 64], BF16, tag=f"hvf_{sd}", name=f"hvf_{sd}")
                for vb in range(3):
                    ps_t = pp1.tile([128, 64], BF16, tag="auxb")
                    nc.tensor.transpose(ps_t[:], h0[:, vb*128:(vb+1)*128], identb[0:64, 0:64])
                    nc.vector.tensor_copy(hvf[:, vb*64:(vb+1)*64], ps_t[:])

                # onehot_src [3][128, 1024] bf16
                oh_src = gwp.tile([128, 3 * 1024], BF16, tag="ohsrc")
                for vb in range(3):
                    nc.vector.tensor_scalar(
                        out=oh_src[:, vb*1024:(vb+1)*1024],
                        in0=gi["src_rep"][:], scalar1=iota_colf[:],
                        scalar2=float(128 * vb), op0=ALU.subtract, op1=ALU.is_equal)

                # h_src [64, 1024]
                h_src = gwp.tile([64, 1024], BF16, tag="hsrc")
                for half in range(2):
                    ps_hs = pp.tile([64, 512], F32, tag="mlpA")
                    for vb in range(3):
                        nc.tensor.matmul(
                            ps_hs[:],
                            hvf[:, vb*64:(vb+1)*64],
                            oh_src[:, vb*1024 + half*512: vb*1024 + (half+1)*512],
                            start=(vb == 0), stop=(vb == 2))
                    nc.vector.tensor_copy(h_src[:, half*512:(half+1)*512], ps_hs[:])

                # msg L1 / L2
                m1 = gwp.tile([64, 1024], BF16, tag="m1")
                for half in range(2):
                    cols = slice(half*512, (half+1)*512)
                    ps_m = pp.tile([64, 512], F32, tag="mlpA")
                    nc.tensor.matmul(ps_m[:], w["gnn_msg_W1h"][:],
                                     h_src[:, cols], start=True, stop=False)
                    nc.tensor.matmul(ps_m[:], w["gnn_msg_W1e"][:],
                                     gi["efT"][:, cols], start=False, stop=True)
                    nc.scalar.activation(m1[:, cols], ps_m[:], AF.Silu,
                                         bias=b["gnn_msg_b1"][:])
                m2 = gwp.tile([64, 1024], BF16, tag="m2")
                for half in range(2):
                    cols = slice(half*512, (half+1)*512)
                    ps_m2 = pp.tile([64, 512], F32, tag="mlpB")
                    nc.tensor.matmul(ps_m2[:], w["gnn_msg_W2"][:], m1[:, cols],
                                     start=True, stop=True)
                    nc.scalar.activation(m2[:, cols], ps_m2[:], AF.Silu,
                                         bias=b["gnn_msg_b2"][:])

                # m^T -> m_ef [128, 8, 64]
                m_ef = gwp.tile([128, 8 * 64], BF16, tag="mef")
                for ec in range(8):
                    ps_t = pp1.tile([128, 64], BF16, tag="auxb")
                    nc.tensor.transpose(ps_t[:], m2[:, ec*128:(ec+1)*128], identb[0:64, 0:64])
                    nc.vector.tensor_copy(m_ef[:, ec*64:(ec+1)*64], ps_t[:])

                # onehot_dst per e-chunk, agg [v, f] in 3 blocks
                oh_dst = gwp.tile([128, 8 * 384], BF16, tag="ohdst")
                for ec in range(8):
                    nc.vector.tensor_scalar(
                        out=oh_dst[:, ec*384:(ec+1)*384],
                        in0=iota384f[:], scalar1=gi["dst_col"][:, ec:ec+1],
                        scalar2=0.0, op0=ALU.subtract, op1=ALU.is_equal)
                aggT = gwp.tile([64, 384], BF16, tag="aggT")
                for vb in range(3):
                    ps_a = pp1.tile([128, 64], F32, tag="aux")
                    for ec in range(8):
                        nc.tensor.matmul(
                            ps_a[:], oh_dst[:, ec*384 + vb*128: ec*384 + (vb+1)*128],
                            m_ef[:, ec*64:(ec+1)*64],
                            start=(ec == 0), stop=(ec == 7))
                    agg_sb = gwp.tile([128, 64], BF16, tag="aggsb")
                    nc.vector.tensor_copy(agg_sb[:], ps_a[:])
                    ps_at = pp1.tile([64, 128], BF16, tag="auxb")
                    nc.tensor.transpose(ps_at[:], agg_sb[:], identb[:])
                    nc.vector.tensor_copy(aggT[:, vb*128:(vb+1)*128], ps_at[:])

                # update
                ps_u = pp1.tile([64, 384], F32, tag="aux")
                nc.tensor.matmul(ps_u[:], w["gnn_upd_Wh"][:], h0[:],
                                 start=True, stop=False)
                nc.tensor.matmul(ps_u[:], w["gnn_upd_Wa"][:], aggT[:],
                                 start=False, stop=True)
                h = cp.tile([64, 384], BF16, tag=f"hfin_{sd}")
                nc.scalar.activation(h[:], ps_u[:], AF.Silu, bias=b["gnn_upd_b"][:])
                return h

            h1 = run_gnn(1)
            h2 = run_gnn(2)

            # femb-L1 projections: blocks [128, 128] bf16 x3 for A1,B1,A2,B2
            projs = {}
            for nm, h, Wp in (("A1", h1, w["femb_W1a"][:]),
                              ("B1", h1, w["femb_W1b"][:]),
                              ("A2", h2, w["femb_W1a"][:]),
                              ("B2", h2, w["femb_W1b"][:])):
                t = cp.tile([128, 3 * 128], BF16, tag=f"proj_{nm}", name=f"proj_{nm}")
                for vb in range(3):
                    ps_p = pp1.tile([128, 128], F32, tag="aux")
                    nc.tensor.matmul(ps_p[:], h[:, vb*128:(vb+1)*128], Wp,
                                     start=True, stop=True)
                    nc.vector.tensor_copy(t[:, vb*128:(vb+1)*128], ps_p[:])
                projs[nm] = t

            gnn_stack.close()
            pipe_stack = ExitStack()
            gp = pipe_stack.enter_context(tc.tile_pool(name="gather", bufs=2))
            gp1 = pipe_stack.enter_context(tc.tile_pool(name="gath1", bufs=1))
            sp = pipe_stack.enter_context(tc.tile_pool(name="scal", bufs=1))

            # segment accumulator [1, 16]
            sgm_ps = pp1.tile([1, 16], F32, tag="sgm")
            n_sgm = [0]

            # ---------------- pair pipeline ----------------
            for s in range(NSC):
                # --- phase A: T gather ---
                tg = gp.tile([128, SLICE], F32, tag="tg")
                nc.gpsimd.ap_gather(
                    out_ap=tg[:], in_ap=ttab_t[:],
                    idxs_ap=tidx_t[:, s*(SLICE//16):(s+1)*(SLICE//16)],
                    channels=128, num_elems=6144, d=1, num_idxs=SLICE)

                # --- phase B: T transpose -> Tscal [128, NGS, 32] f32 ---
                Tscal = gp1.tile([128, NGS * 32], F32, tag="Tscal")
                Tv = Tscal[:].rearrange("p (g f) -> p g f", f=32)
                for l in range(NSLICE):
                    for wq in range(SLICE // 128 // 4):   # 8 psum batches of 4
                        ps_tt = pp1.tile([128, 128], F32, tag="aux")
                        for k in range(4):
                            wcol = wq * 4 + k
                            nc.tensor.transpose(
                                ps_tt[:, k*32:(k+1)*32],
                                tg[32*l:32*l+32, wcol*128:(wcol+1)*128],
                                ident[32*l:32*l+32, 32*l:32*l+32],
                                tile_position=(32*l, 0))
                        g0 = l * (SLICE // 128) + wq * 4
                        nc.vector.tensor_copy(
                            Tscal[:, g0*32:(g0+4)*32], ps_tt[:])

                # --- phase C: scalar land ---
                def T1f(f0, n=1):
                    return Tv[:, :, f0:f0+n]

                def T2f(f0, n=1):
                    return Tv[:, :, 16+f0:16+f0+n]

                def stile(name, k, dt=F32):
                    return sp.tile([128, NGS * k], dt, tag=name, name=name)[:].rearrange(
                        "p (g f) -> p g f", f=k)

                Dif = stile("Dif", 3)
                nc.vector.tensor_tensor(out=Dif, in0=T2f(13, 3), in1=T1f(13, 3),
                                        op=ALU.subtract)
                sq = stile("sq", 3)
                nc.vector.tensor_tensor(out=sq, in0=Dif, in1=Dif, op=ALU.mult)
                R2 = stile("R2", 1)
                nc.vector.tensor_reduce(out=R2, in_=sq, axis=AX.X, op=ALU.add)
                invR2 = stile("invR2", 1)
                nc.vector.reciprocal(invR2, R2)
                invR1 = stile("invR1", 1)
                nc.scalar.activation(invR1, invR2, AF.Sqrt)
                R1 = stile("R1", 1)
                nc.vector.tensor_tensor(out=R1, in0=R2, in1=invR1, op=ALU.mult)
                Rx = stile("Rx", 3)
                nc.vector.tensor_tensor(out=Rx, in0=Dif,
                                        in1=invR1.to_broadcast([128, NGS, 3]),
                                        op=ALU.mult)

                def dot3(name, a, bb):
                    t = stile(name + "_t", 3)
                    nc.vector.tensor_tensor(out=t, in0=a, in1=bb, op=ALU.mult)
                    r = stile(name, 1)
                    nc.vector.tensor_reduce(out=r, in_=t, axis=AX.X, op=ALU.add)
                    return r

                D1 = dot3("D1", T1f(1, 3), Rx)
                D2 = dot3("D2", T2f(1, 3), Rx)
                dd = dot3("dd", T1f(1, 3), T2f(1, 3))
                # Q1R / Q2R: [128, g, 3k] = sum_j q[3k+j]*Rx[j]
                q1Rt = sp.tile([128, NGS * 9], F32, tag="scr9", name="q1Rt")[:].rearrange(
                    "p (g k j) -> p g k j", k=3, j=3)
                Rxb = Rx.rearrange("p g (o j) -> p g o j", o=1)
                nc.vector.tensor_tensor(out=q1Rt,
                                        in0=Tv[:, :, 4:13].rearrange("p g (k j) -> p g k j", k=3),
                                        in1=Rxb.to_broadcast([128, NGS, 3, 3]),
                                        op=ALU.mult)
                Q1R = stile("Q1R", 3)
                nc.vector.tensor_reduce(out=Q1R, in_=q1Rt, axis=AX.X, op=ALU.add)
                q2Rt = sp.tile([128, NGS * 9], F32, tag="scr9", name="q2Rt")[:].rearrange(
                    "p (g k j) -> p g k j", k=3, j=3)
                nc.vector.tensor_tensor(out=q2Rt,
                                        in0=Tv[:, :, 20:29].rearrange("p g (k j) -> p g k j", k=3),
                                        in1=Rxb.to_broadcast([128, NGS, 3, 3]),
                                        op=ALU.mult)
                Q2R = stile("Q2R", 3)
                nc.vector.tensor_reduce(out=Q2R, in_=q2Rt, axis=AX.X, op=ALU.add)
                Q1RR = dot3("Q1RR", Q1R, Rx)
                Q2RR = dot3("Q2RR", Q2R, Rx)
                QRd2 = dot3("QRd2", Q1R, T2f(1, 3))
                QRd1 = dot3("QRd1", Q2R, T1f(1, 3))
                QRQR = dot3("QRQR", Q1R, Q2R)
                qqt = sp.tile([128, NGS * 9], F32, tag="scr9", name="qqt")[:].rearrange(
                    "p (g f) -> p g f", f=9)
                nc.vector.tensor_tensor(out=qqt, in0=Tv[:, :, 4:13],
                                        in1=Tv[:, :, 20:29], op=ALU.mult)
                qq = stile("qq", 1)
                nc.vector.tensor_reduce(out=qq, in_=qqt, axis=AX.X, op=ALU.add)

                # switch
                xsw = stile("xsw", 1)
                nc.vector.tensor_scalar(out=xsw, in0=R1, scalar1=-9.0, scalar2=0.0,
                                        op0=ALU.add, op1=ALU.max)
                nc.vector.tensor_scalar(out=xsw, in0=xsw, scalar1=1.0, scalar2=None,
                                        op0=ALU.min)
                x2 = stile("x2", 1)
                nc.vector.tensor_tensor(out=x2, in0=xsw, in1=xsw, op=ALU.mult)
                x3 = stile("x3", 1)
                nc.vector.tensor_tensor(out=x3, in0=x2, in1=xsw, op=ALU.mult)
                pa = stile("pa", 1)
                nc.vector.tensor_scalar(out=pa, in0=xsw, scalar1=6.0, scalar2=-15.0,
                                        op0=ALU.mult, op1=ALU.add)
                nc.vector.tensor_tensor(out=pa, in0=pa, in1=xsw, op=ALU.mult)
                nc.vector.tensor_scalar(out=pa, in0=pa, scalar1=10.0, scalar2=None,
                                        op0=ALU.add)
                swt = stile("swt", 1)
                nc.vector.tensor_tensor(out=swt, in0=pa, in1=x3, op=ALU.mult)
                sw = stile("sw", 1)
                nc.vector.tensor_scalar(out=sw, in0=swt, scalar1=-1.0, scalar2=1.0,
                                        op0=ALU.mult, op1=ALU.add)

                # G bundle [128, NGS, 16] bf16
                Gb = sp.tile([128, NGS * 16], BF16, tag="Gb", name="Gb")[:].rearrange(
                    "p (g f) -> p g f", f=16)

                def gcol(f0, n=1):
                    return Gb[:, :, f0:f0+n]

                nc.vector.tensor_tensor(out=gcol(0), in0=T1f(0), in1=T2f(0), op=ALU.mult)
                t1 = stile("gt1", 1)
                nc.vector.tensor_tensor(out=t1, in0=D1, in1=T2f(0), op=ALU.mult)
                t2 = stile("gt2", 1)
                nc.vector.tensor_tensor(out=t2, in0=D2, in1=T1f(0), op=ALU.mult)
                nc.vector.tensor_tensor(out=gcol(1), in0=t1, in1=t2, op=ALU.subtract)
                nc.vector.tensor_copy(gcol(2), dd)
                nc.vector.tensor_tensor(out=t1, in0=D1, in1=D2, op=ALU.mult)
                nc.vector.tensor_scalar(out=gcol(3), in0=t1, scalar1=-1.0, scalar2=None,
                                        op0=ALU.mult)
                nc.vector.tensor_tensor(out=t1, in0=QRd2, in1=QRd1, op=ALU.subtract)
                nc.vector.tensor_scalar(out=gcol(4), in0=t1, scalar1=2.0, scalar2=None,
                                        op0=ALU.mult)
                nc.vector.tensor_tensor(out=t1, in0=Q1RR, in1=T2f(0), op=ALU.mult)
                nc.vector.tensor_tensor(out=t2, in0=Q2RR, in1=T1f(0), op=ALU.mult)
                nc.vector.tensor_tensor(out=gcol(5), in0=t1, in1=t2, op=ALU.add)
                nc.vector.tensor_scalar(out=gcol(6), in0=qq, scalar1=2.0, scalar2=None,
                                        op0=ALU.mult)
                nc.vector.tensor_scalar(out=gcol(7), in0=QRQR, scalar1=-4.0, scalar2=None,
                                        op0=ALU.mult)
                nc.vector.tensor_tensor(out=t1, in0=Q2RR, in1=D1, op=ALU.mult)
                nc.vector.tensor_tensor(out=t2, in0=Q1RR, in1=D2, op=ALU.mult)
                nc.vector.tensor_tensor(out=gcol(8), in0=t1, in1=t2, op=ALU.subtract)
                nc.vector.tensor_tensor(out=gcol(9), in0=Q1RR, in1=Q2RR, op=ALU.mult)

                # dist feats: exp(-gamma_k R2) * sw
                for k in range(5):
                    nc.scalar.activation(Gb[:, :, 10+k:11+k], R2, AF.Exp,
                                         scale=float(-GAMMAS[k]))
                nc.vector.tensor_tensor(out=Gb[:, :, 10:15], in0=Gb[:, :, 10:15],
                                        in1=sw.to_broadcast([128, NGS, 5]), op=ALU.mult)
                nc.gpsimd.memset(Gb[:, :, 15:16], 0.0)

                # --- phase D: MLPs per chunk of 512 ---
                SPacc_flat = sp.tile([128, NGS * 5], F32, tag="SPacc")
                SPacc = SPacc_flat[:].rearrange("p (g f) -> p g f", f=5)
                for ch in range(NCHS):
                    q0 = s * SC + ch * 512
                    iis = ip.tile([128, 512], FP16, tag="iis")
                    nc.sync.dma_start(iis[:], ii_rep[:, q0:q0+512])
                    ijs = ip.tile([128, 512], FP16, tag="ijs")
                    nc.sync.dma_start(ijs[:], ij_rep[:, q0:q0+512])
                    oh1 = wp.tile([128, 3 * 512], BF16, tag="oh1")
                    oh2 = wp.tile([128, 3 * 512], BF16, tag="oh2")
                    for vb in range(3):
                        nc.vector.tensor_scalar(
                            out=oh1[:, vb*512:(vb+1)*512], in0=iis[:],
                            scalar1=iota_colf[:], scalar2=float(128*vb),
                            op0=ALU.subtract, op1=ALU.is_equal)
                        nc.vector.tensor_scalar(
                            out=oh2[:, vb*512:(vb+1)*512], in0=ijs[:],
                            scalar1=iota_colf[:], scalar2=float(128*vb),
                            op0=ALU.subtract, op1=ALU.is_equal)

                    psA = pp.tile([128, 512], F32, tag="mlpA")
                    psB = pp.tile([128, 512], F32, tag="mlpB")
                    for vb in range(3):
                        vs = slice(vb*512, (vb+1)*512)
                        bs = slice(vb*128, (vb+1)*128)
                        nc.tensor.matmul(psA[:], projs["A1"][:, bs], oh1[:, vs],
                                         start=(vb == 0), stop=False)
                        nc.tensor.matmul(psA[:], projs["B2"][:, bs], oh2[:, vs],
                                         start=False, stop=(vb == 2))
                        nc.tensor.matmul(psB[:], projs["A2"][:, bs], oh2[:, vs],
                                         start=(vb == 0), stop=False)
                        nc.tensor.matmul(psB[:], projs["B1"][:, bs], oh1[:, vs],
                                         start=False, stop=(vb == 2))
                    l1a = wp.tile([128, 512], BF16, tag="l1a")
                    nc.scalar.activation(l1a[:], psA[:], AF.Silu, bias=b["femb_b1"][:])
                    l1b = wp.tile([128, 512], BF16, tag="l1b")
                    nc.scalar.activation(l1b[:], psB[:], AF.Silu, bias=b["femb_b1"][:])

                    ps2a = pp.tile([128, 512], F32, tag="mlpA")
                    nc.tensor.matmul(ps2a[:], w["femb_W2"][:], l1a[:], start=True, stop=True)
                    l2a = wp.tile([128, 512], BF16, tag="l2a")
                    nc.scalar.activation(l2a[:], ps2a[:], AF.Silu, bias=b["femb_b2"][:])
                    ps2b = pp.tile([128, 512], F32, tag="mlpB")
                    nc.tensor.matmul(ps2b[:], w["femb_W2"][:], l1b[:], start=True, stop=True)
                    l2b = wp.tile([128, 512], BF16, tag="l2b")
                    nc.scalar.activation(l2b[:], ps2b[:], AF.Silu, bias=b["femb_b2"][:])

                    ps3 = pp.tile([128, 512], F32, tag="mlpA")
                    nc.tensor.matmul(ps3[:], w["femb_W3"][:], l2a[:], start=True, stop=False)
                    nc.tensor.matmul(ps3[:], w["femb_W3"][:], l2b[:], start=False, stop=True)
                    nf = wp.tile([128, 512], BF16, tag="nf")
                    nc.scalar.activation(nf[:], ps3[:], AF.Identity, bias=b["femb_b3"][:])

                    # G rows for chunk: [16, 512] bf16
                    ps_g = pp1.tile([16, 512], BF16, tag="auxb")
                    for k in range(4):
                        gma = ch * 4 + k
                        nc.tensor.transpose(ps_g[:, k*128:(k+1)*128],
                                            Gb[:, gma, :], identb[:])
                    Gch = wp.tile([16, 512], BF16, tag="Gch")
                    nc.vector.tensor_copy(Gch[:], ps_g[:])

                    ps_s = pp.tile([128, 512], F32, tag="mlpB")
                    nc.tensor.matmul(ps_s[:], w["S_W1_nf"][:], nf[:], start=True, stop=False)
                    nc.tensor.matmul(ps_s[:], w["S_W1_G"][:], Gch[:], start=False, stop=True)
                    sh1 = wp.tile([128, 512], BF16, tag="sh1")
                    nc.scalar.activation(sh1[:], ps_s[:], AF.Silu, bias=b["S_b1"][:])
                    ps_s2 = pp.tile([128, 512], F32, tag="mlpA")
                    nc.tensor.matmul(ps_s2[:], w["S_W2"][:], sh1[:], start=True, stop=True)
                    sh2 = wp.tile([128, 512], BF16, tag="sh2")
                    nc.scalar.activation(sh2[:], ps_s2[:], AF.Silu, bias=b["S_b2"][:])

                    ps_k = pp.tile([128, 512], F32, tag="mlpB")
                    nc.tensor.matmul(ps_k[:], w["K_W1_nf"][:], nf[:], start=True, stop=False)
                    nc.tensor.matmul(ps_k[:], w["K_W1_G"][:], Gch[:], start=False, stop=True)
                    kh1 = wp.tile([128, 512], BF16, tag="kh1")
                    nc.scalar.activation(kh1[:], ps_k[:], AF.Silu, bias=b["K_b1"][:])
                    ps_k2 = pp.tile([128, 512], F32, tag="mlpA")
                    nc.tensor.matmul(ps_k2[:], w["K_W2"][:], kh1[:], start=True, stop=True)
                    kh2 = wp.tile([128, 512], BF16, tag="kh2")
                    nc.scalar.activation(kh2[:], ps_k2[:], AF.Silu, bias=b["K_b2"][:])

                    # final linears: S3 rows 0:2, K3 rows 32:35 of one psum tile
                    ps_f = pp1.tile([35, 512], F32, tag="fin")
                    nc.tensor.matmul(ps_f[0:2, :], w["S_W3"][:], sh2[:], start=True, stop=True)
                    nc.tensor.matmul(ps_f[32:35, :], w["K_W3"][:], kh2[:], start=True, stop=True)
                    f_sb = wp.tile([35, 512], F32, tag="fsb")
                    nc.vector.tensor_copy(f_sb[0:2, :], ps_f[0:2, :])
                    nc.vector.tensor_copy(f_sb[32:35, :], ps_f[32:35, :])
                    # add biases (S_b3 [2,1], K_b3 [3,1])
                    nc.vector.tensor_scalar(out=f_sb[0:2, :], in0=f_sb[0:2, :],
                                            scalar1=b["S_b3"][:], scalar2=None,
                                            op0=ALU.add)
                    nc.vector.tensor_scalar(out=f_sb[32:35, :], in0=f_sb[32:35, :],
                                            scalar1=b["K_b3"][:], scalar2=None,
                                            op0=ALU.add)
                    # transpose to scalar land [128, 35] per gamma
                    ps_v = pp1.tile([128, 140], F32, tag="aux")
                    for k in range(4):
                        gma = ch * 4 + k
                        nc.tensor.transpose(ps_v[:, k*35:(k+1)*35],
                                            f_sb[:, k*128:(k+1)*128], ident[0:35, 0:35])
                    for k in range(4):
                        gma = ch * 4 + k
                        nc.vector.tensor_copy(SPacc[:, gma, 0:2], ps_v[:, k*35:k*35+2])
                        nc.vector.tensor_copy(SPacc[:, gma, 2:5], ps_v[:, k*35+32:k*35+35])

                # --- phase E: softplus + V + segment ---
                spv_flat = sp.tile([128, NGS * 5], F32, tag="spv")
                spv = spv_flat[:].rearrange("p (g f) -> p g f", f=5)
                # softplus(x) = ln(1 + exp(x)); Exp and Ln share one table set
                nc.scalar.activation(spv_flat[:], SPacc_flat[:], AF.Exp)
                nc.vector.tensor_scalar(out=spv_flat[:], in0=spv_flat[:],
                                        scalar1=1.0, scalar2=None, op0=ALU.add)
                nc.scalar.activation(spv_flat[:], spv_flat[:], AF.Ln)
                S2s = stile("S2s", 1)
                nc.vector.tensor_tensor(out=S2s, in0=spv[:, :, 0:1], in1=sw, op=ALU.mult)
                S2at = stile("S2at", 1)
                nc.vector.tensor_tensor(out=S2at, in0=spv[:, :, 1:2], in1=sw, op=ALU.mult)
                # V = K1*S2s*invR1 + K2*S2s*invR2 - Kat*S2at
                va = stile("va", 1)
                nc.vector.tensor_tensor(out=va, in0=spv[:, :, 2:3], in1=invR1, op=ALU.mult)
                nc.vector.tensor_tensor(out=va, in0=va, in1=S2s, op=ALU.mult)
                vbt = stile("vbt", 1)
                nc.vector.tensor_tensor(out=vbt, in0=spv[:, :, 3:4], in1=invR2, op=ALU.mult)
                nc.vector.tensor_tensor(out=vbt, in0=vbt, in1=S2s, op=ALU.mult)
                nc.vector.tensor_tensor(out=va, in0=va, in1=vbt, op=ALU.add)
                nc.vector.tensor_tensor(out=vbt, in0=spv[:, :, 4:5], in1=S2at, op=ALU.mult)
                Vt = sp.tile([128, NGS], BF16, tag="Vt")
                nc.vector.tensor_tensor(out=Vt[:].rearrange("p (g f) -> p g f", f=1),
                                        in0=va, in1=vbt, op=ALU.subtract)

                # onehot_b [128, NGS, 16] bf16
                ohb = sp.tile([128, NGS * 16], BF16, tag="ohb", name="ohb")[:].rearrange(
                    "p (g k) -> p g k", k=16)
                nc.vector.tensor_tensor(
                    out=ohb,
                    in0=ib_t[:, s*NGS:(s+1)*NGS].rearrange("p (g k) -> p g k", k=1)
                        .to_broadcast([128, NGS, 16]),
                    in1=iota16f[:].rearrange("p (g k) -> p g k", g=1)
                        .to_broadcast([128, NGS, 16]),
                    op=ALU.is_equal)

                for gma in range(NGS):
                    last = (s == NSC - 1) and (gma == NGS - 1)
                    nc.tensor.matmul(
                        sgm_ps[:], Vt[:, gma:gma+1],
                        ohb[:, gma, :],
                        start=(n_sgm[0] == 0), stop=last, skip_group_check=True)
                    n_sgm[0] += 1

            out_sb = cp.tile([1, 16], F32)
            nc.vector.tensor_copy(out_sb[:], sgm_ps[:])
            nc.sync.dma_start(out_d[:], out_sb[:])
            pipe_stack.close()

    nc.compile()
    return nc


# ---------------------------------------------------------------------------
# PJRT runner (inlined for self-containment)
# ---------------------------------------------------------------------------

class _SpmdRunner:
    def __init__(self, nc, n_cores=8):
        import jax
        from jax.sharding import Mesh, PartitionSpec
        from jax.experimental.shard_map import shard_map
        from concourse import mybir
        from concourse.bass2jax import (_bass_exec_p, install_neuronx_cc_hook,
                                        partition_id_tensor)
        install_neuronx_cc_hook()
        self.jax = jax
        self.n_cores = n_cores
        partition_name = nc.partition_id_tensor.name if nc.partition_id_tensor else None
        in_names, out_names, out_avals, zero_outs = [], [], [], []
        for alloc in nc.m.functions[0].allocations:
            if not isinstance(alloc, mybir.MemoryLocationSet):
                continue
            name = alloc.memorylocations[0].name
            if alloc.kind == "ExternalInput":
                if name != partition_name:
                    in_names.append(name)
            elif alloc.kind == "ExternalOutput":
                out_names.append(name)
                shape = tuple(alloc.tensor_shape)
                dtype = mybir.dt.np(alloc.dtype)
                out_avals.append(jax.core.ShapedArray(shape, dtype))
                zero_outs.append(np.zeros(shape, dtype))
        self.in_names, self.out_names = in_names, out_names
        self.out_avals, self.zero_outs = out_avals, zero_outs
        n_params, n_outs = len(in_names), len(out_avals)
        self.n_params = n_params

        all_in_names = list(in_names) + list(out_names)
        if partition_name is not None:
            all_in_names.append(partition_name)

        def _body(*args):
            operands = list(args)
            if partition_name is not None:
                operands.append(partition_id_tensor())
            outs = _bass_exec_p.bind(
                *operands,
                out_avals=tuple(out_avals),
                in_names=tuple(all_in_names),
                out_names=tuple(out_names),
                lowering_input_output_aliases=(),
                sim_require_finite=False,
                sim_require_nnan=False,
                nc=nc,
            )
            return tuple(outs)

        devices = jax.devices()[:n_cores]
        mesh = Mesh(np.asarray(devices), ("core",))
        n_all = n_params + n_outs
        self.fn = jax.jit(
            shard_map(_body, mesh=mesh,
                      in_specs=(PartitionSpec("core"),) * n_all,
                      out_specs=(PartitionSpec("core"),) * n_outs,
                      check_rep=False),
            keep_unused=True,
        )

    def prepare(self, in_maps):
        jax = self.jax
        per_core = [[np.asarray(m[n]) for n in self.in_names] for m in in_maps]
        concat_in = [
            np.concatenate([per_core[c][i] for c in range(self.n_cores)], axis=0)
            for i in range(self.n_params)
        ]
        concat_zeros = [
            np.zeros((self.n_cores * z.shape[0], *z.shape[1:]), z.dtype)
            for z in self.zero_outs
        ]
        self.args = [jax.device_put(a) for a in (concat_in + concat_zeros)]
        return self

    def run(self):
        jax = self.jax
        outs = self.fn(*self.args)
        jax.block_until_ready(outs)
        return [
            {
                name: np.asarray(outs[i]).reshape(self.n_cores, *self.out_avals[i].shape)[c]
                for i, name in enumerate(self.out_names)
            }
            for c in range(self.n_cores)
        ]


def _get_runner():
    global _RUNNER
    if _RUNNER is None:
        nc = _build_nc()
        _RUNNER = _SpmdRunner(nc, 8)
    return _RUNNER


# ---------------------------------------------------------------------------
# public entry
# ---------------------------------------------------------------------------

def kernel(**inputs):
    runner = _get_runner()
    cores = _host_prep(inputs)
    runner.prepare(cores)
    results = runner.run()
    out = np.zeros(16, np.float32)
    for c in range(8):
        out += results[c]["out"].reshape(16)
    return out
